# revision 5
# baseline (speedup 1.0000x reference)
"""Multi-head masked attention (B=2, T=2048, C=2048, past=1024, H=16) on 8 Trainium2
NeuronCores.

Sharding: core = (batch b, group of 4 heads). Each core computes its heads' QKV
projection, causal attention, and a partial output projection (W_o row-shard);
the host sums the 4 partial y's per batch and assembles the returned k/v caches.

All matmuls run on the tensor engine in fp32r (TF32-like: 11-bit mantissa,
fp32 accumulate). Matmul operands are pre-rounded to fp32r on the host (DRAM
inputs) or rounded on write by the producing engine (fp32r-typed SBUF tiles).

Device layouts (per core):
  xT   [C, T]        x[b] transposed              (host pre-transposes)
  qT   [D*4, T]      q transposed, scale folded in
  kT   [D*4, S]      keys transposed (past ++ new)
  v    [S, D*4]      values natural (past ++ new)
  scoresT tile [s:128, t:512] = kT_chunk.T @ qT_tile   (PSUM)
  expT = exp(scoresT), causal-masked via gpsimd affine_select on diagonal tiles
  yhT  [D, t] = v_chunk.T @ expT (accum over s-chunks), divided by the
        column-sum of expT obtained from a ones[128,128] matmul on the same expT
  yT_partial [C, T] = Wo_block.T @ attn_outT (accum over the 4 head chunks)
"""

import math
import os
import sys

sys.path.insert(0, "/opt/trn_rl_repo")

os.environ.setdefault("JAX_COMPILATION_CACHE_DIR", "/root/.cache/jax_comp_mhma")

import numpy as np

import concourse.bass as bass  # noqa: F401  (engine classes referenced via nc)
import concourse.mybir as mybir
import concourse.tile as tile
from concourse import bacc
from concourse.bass_utils import run_bass_kernel_spmd

F32 = mybir.dt.float32
F32R = mybir.dt.float32r
AF = mybir.ActivationFunctionType

B, T, C, PT = 2, 2048, 2048, 1024
H_TOT, D, P = 16, 128, 128
S = PT + T              # 3072
HG = 4                  # heads per core
GC = HG * D             # 512 cols per core
NCORES = 8
CC = C // P             # 16 contraction chunks
HALF = T // 2           # 1024
JT = 512                # T tile (matmul free dim)
NSC = S // P            # 24 key chunks
SCALE = 1.0 / math.sqrt(D)


def _round_fp32r(a: np.ndarray) -> np.ndarray:
    """RNE-round fp32 to fp32r (11-bit mantissa; low 12 bits zeroed)."""
    a = np.ascontiguousarray(a, dtype=np.float32)
    u = a.view(np.uint32)
    r = (u + np.uint32(0x7FF) + ((u >> np.uint32(12)) & np.uint32(1))) & np.uint32(
        0xFFFFF000
    )
    return r.view(np.float32)


def _build():
    nc = bacc.Bacc("TRN2", target_bir_lowering=False, debug=False, num_devices=NCORES)

    xT = nc.dram_tensor("xT", [C, T], F32R, kind="ExternalInput")
    wq = nc.dram_tensor("wq", [C, GC], F32R, kind="ExternalInput")
    wk = nc.dram_tensor("wk", [C, GC], F32R, kind="ExternalInput")
    wv = nc.dram_tensor("wv", [C, GC], F32R, kind="ExternalInput")
    wo = nc.dram_tensor("wo", [GC, C], F32R, kind="ExternalInput")
    pkt = nc.dram_tensor("pkt", [HG, P, PT], F32R, kind="ExternalInput")
    pvn = nc.dram_tensor("pvn", [PT, GC], F32R, kind="ExternalInput")
    bqs = nc.dram_tensor("bqs", [P, HG], F32, kind="ExternalInput")
    bks = nc.dram_tensor("bks", [P, HG], F32, kind="ExternalInput")
    bvt = nc.dram_tensor("bvt", [P, GC], F32, kind="ExternalInput")
    ones = nc.dram_tensor("ones", [P, P], F32R, kind="ExternalInput")

    yt = nc.dram_tensor("yt", [C, T], F32, kind="ExternalOutput")
    ktn = nc.dram_tensor("ktn", [HG, P, T], F32, kind="ExternalOutput")
    vn = nc.dram_tensor("vn", [T, GC], F32, kind="ExternalOutput")

    with tile.TileContext(nc) as tc:
        with (
            tc.tile_pool(name="big", bufs=1) as bigp,      # xT half / attn_outT (shared slot)
            tc.tile_pool(name="ktp", bufs=1) as ktp,
            tc.tile_pool(name="vsp", bufs=1) as vsp,
            tc.tile_pool(name="qtp", bufs=1) as qtp,
            tc.tile_pool(name="wblk", bufs=2) as wblkp,
            tc.tile_pool(name="wvp", bufs=2) as wvp,
            tc.tile_pool(name="expp", bufs=4) as expp,
            tc.tile_pool(name="work", bufs=2) as workp,
            tc.tile_pool(name="consts", bufs=1) as consts,
            tc.tile_pool(name="pp", bufs=8, space="PSUM") as pp,
        ):
            kT = ktp.tile([P, HG, S], F32R, tag="kt")
            vS = vsp.tile([P, NSC, GC], F32R, tag="vs")

            bqs_sb = consts.tile([P, HG], F32, tag="bqs")
            bks_sb = consts.tile([P, HG], F32, tag="bks")
            bvt_sb = consts.tile([P, GC], F32, tag="bvt")
            ones_sb = consts.tile([P, P], F32R, tag="ones")
            nc.sync.dma_start(out=bqs_sb, in_=bqs.ap())
            nc.sync.dma_start(out=bks_sb, in_=bks.ap())
            nc.sync.dma_start(out=bvt_sb, in_=bvt.ap())
            nc.sync.dma_start(out=ones_sb, in_=ones.ap())

            # past K (transposed) and past V (natural) into the caches
            for h in range(HG):
                nc.sync.dma_start(out=kT[:, h, 0:PT], in_=pkt.ap()[h])
            for sc in range(PT // P):
                nc.sync.dma_start(
                    out=vS[:, sc, :], in_=pvn.ap()[sc * P:(sc + 1) * P, :]
                )

            for Hh in range(2):  # T halves
                tbase = Hh * HALF

                # ---- load xT half: [P, CC, HALF], one DMA per contraction chunk
                xa = bigp.tile([P, CC, HALF], F32R, tag="big")
                xsrc = xT.ap()[:, tbase:tbase + HALF].rearrange(
                    "(cc p) t -> p cc t", p=P
                )
                for c in range(CC):
                    nc.sync.dma_start(out=xa[:, c, :], in_=xsrc[:, c, :])

                qT = qtp.tile([P, HG, HALF], F32R, tag="qt")

                # ---- Q projection: qT[d, h, t] = (Wq.T @ xT) * SCALE + bq*SCALE
                for h in range(HG):
                    wsrc = wq.ap()[:, h * P:(h + 1) * P].rearrange(
                        "(cc p) m -> p cc m", p=P
                    )
                    blks = []
                    for hb in range(2):
                        blk = wblkp.tile([P, 8, P], F32R, tag="wblk",
                                         name=f"wqb_{Hh}_{h}_{hb}")
                        for qq in range(2):
                            nc.sync.dma_start(
                                out=blk[:, 4 * qq:4 * qq + 4, :],
                                in_=wsrc[:, 8 * hb + 4 * qq:8 * hb + 4 * qq + 4, :],
                            )
                        blks.append(blk)
                    for jl in range(2):
                        ps = pp.tile([P, JT], F32, tag="ps")
                        for c in range(CC):
                            nc.tensor.matmul(
                                ps,
                                blks[c // 8][:, c % 8, :],
                                xa[:, c, jl * JT:(jl + 1) * JT],
                                start=(c == 0),
                                stop=(c == CC - 1),
                            )
                        nc.scalar.activation(
                            out=qT[:, h, jl * JT:(jl + 1) * JT],
                            in_=ps,
                            func=AF.Identity,
                            bias=bqs_sb[:, h:h + 1],
                            scale=SCALE,
                        )

                # ---- K projection: kT[d, h, PT + t] = Wk.T @ xT + bk
                for h in range(HG):
                    wsrc = wk.ap()[:, h * P:(h + 1) * P].rearrange(
                        "(cc p) m -> p cc m", p=P
                    )
                    blks = []
                    for hb in range(2):
                        blk = wblkp.tile([P, 8, P], F32R, tag="wblk",
                                         name=f"wkb_{Hh}_{h}_{hb}")
                        for qq in range(2):
                            nc.sync.dma_start(
                                out=blk[:, 4 * qq:4 * qq + 4, :],
                                in_=wsrc[:, 8 * hb + 4 * qq:8 * hb + 4 * qq + 4, :],
                            )
                        blks.append(blk)
                    for jl in range(2):
                        ps = pp.tile([P, JT], F32, tag="ps")
                        for c in range(CC):
                            nc.tensor.matmul(
                                ps,
                                blks[c // 8][:, c % 8, :],
                                xa[:, c, jl * JT:(jl + 1) * JT],
                                start=(c == 0),
                                stop=(c == CC - 1),
                            )
                        nc.scalar.activation(
                            out=kT[:, h, PT + tbase + jl * JT: PT + tbase + (jl + 1) * JT],
                            in_=ps,
                            func=AF.Identity,
                            bias=bks_sb[:, h:h + 1],
                            scale=1.0,
                        )
                    # new-K cache slice out (fp32r bits are valid fp32)
                    nc.sync.dma_start(
                        out=ktn.ap()[h, :, tbase:tbase + HALF],
                        in_=kT[:, h, PT + tbase: PT + tbase + HALF].bitcast(F32),
                    )

                # ---- V projection: v[t, d] = xT.T @ Wv + bv  (8 PSUM groups, c outer)
                vps = [pp.tile([P, JT], F32, tag="ps", name=f"vps_{Hh}_{m}") for m in range(8)]
                for c in range(CC):
                    wvt = wvp.tile([P, GC], F32R, tag="wv")
                    nc.sync.dma_start(out=wvt, in_=wv.ap()[c * P:(c + 1) * P, :])
                    for m in range(8):
                        nc.tensor.matmul(
                            vps[m],
                            xa[:, c, m * P:(m + 1) * P],
                            wvt,
                            start=(c == 0),
                            stop=(c == CC - 1),
                        )
                for m in range(8):
                    sc_i = PT // P + Hh * 8 + m
                    nc.vector.tensor_add(
                        out=vS[:, sc_i, :], in0=vps[m], in1=bvt_sb
                    )
                    nc.sync.dma_start(
                        out=vn.ap()[tbase + m * P: tbase + (m + 1) * P, :],
                        in_=vS[:, sc_i, :].bitcast(F32),
                    )

                # ---- attention for this half's queries
                attn = bigp.tile([P, HG, HALF], F32R, tag="big")
                for jl in range(2):
                    jg = 2 * Hh + jl
                    t0 = jg * JT
                    n_s = 12 + 4 * jg          # valid key chunks (causal skip)
                    diag_lo = 8 + 4 * jg       # first diagonal-crossing chunk
                    for h in range(HG):
                        ps_pv = pp.tile([P, JT], F32, tag="ps")
                        ps_den = pp.tile([P, JT], F32, tag="ps")
                        for i in range(n_s):
                            ps_sc = pp.tile([P, JT], F32, tag="ps")
                            nc.tensor.matmul(
                                ps_sc,
                                kT[:, h, i * P:(i + 1) * P],
                                qT[:, h, jl * JT:(jl + 1) * JT],
                                start=True,
                                stop=True,
                            )
                            et = expp.tile([P, JT], F32R, tag="et")
                            nc.scalar.activation(out=et, in_=ps_sc, func=AF.Exp)
                            if i >= diag_lo:
                                # keep where key_pos <= query_pos:
                                # (PT + t0 + f) - (i*P + p) >= 0
                                nc.gpsimd.affine_select(
                                    out=et,
                                    in_=et,
                                    pattern=[[1, JT]],
                                    base=PT + t0 - i * P,
                                    channel_multiplier=-1,
                                    compare_op=mybir.AluOpType.is_ge,
                                    fill=0.0,
                                )
                            nc.tensor.matmul(
                                ps_pv,
                                vS[:, i, h * P:(h + 1) * P],
                                et,
                                start=(i == 0),
                                stop=(i == n_s - 1),
                            )
                            nc.tensor.matmul(
                                ps_den,
                                ones_sb,
                                et,
                                start=(i == 0),
                                stop=(i == n_s - 1),
                            )
                        rc = workp.tile([P, JT], F32, tag="scratch", name=f"rc_{jg}_{h}")
                        nc.vector.reciprocal(out=rc, in_=ps_den)
                        nc.vector.tensor_mul(
                            out=attn[:, h, jl * JT:(jl + 1) * JT],
                            in0=ps_pv,
                            in1=rc,
                        )

                # ---- partial output projection: yT += Wo_rows.T @ attn_outT
                for co in range(CC):
                    oblk = wblkp.tile([P, HG, P], F32R, tag="wblk", name=f"wob_{Hh}_{co}")
                    osrc = wo.ap()[:, co * P:(co + 1) * P].rearrange(
                        "(hc p) n -> p hc n", p=P
                    )
                    nc.sync.dma_start(out=oblk, in_=osrc)
                    for jl in range(2):
                        ps = pp.tile([P, JT], F32, tag="ps")
                        for hc in range(HG):
                            nc.tensor.matmul(
                                ps,
                                oblk[:, hc, :],
                                attn[:, hc, jl * JT:(jl + 1) * JT],
                                start=(hc == 0),
                                stop=(hc == HG - 1),
                            )
                        ys = workp.tile([P, JT], F32, tag="scratch", name=f"ys_{Hh}_{co}_{jl}")
                        nc.vector.tensor_copy(out=ys, in_=ps)
                        nc.sync.dma_start(
                            out=yt.ap()[co * P:(co + 1) * P,
                                        tbase + jl * JT: tbase + (jl + 1) * JT],
                            in_=ys,
                        )

    nc.compile()
    return nc


_NC = None


def _get_nc():
    global _NC
    if _NC is None:
        _NC = _build()
    return _NC


def _prep_inputs(x, past_k, past_v, W_qkv, b_qkv, W_o, b_o):
    xTr = [_round_fp32r(np.ascontiguousarray(x[b].T)) for b in range(B)]
    ones = np.ones((P, P), np.float32)
    in_maps = []
    percore_hg = {}
    for hg in range(HG):
        cols = slice(hg * GC, (hg + 1) * GC)
        percore_hg[hg] = dict(
            wq=_round_fp32r(W_qkv[:, :C][:, cols]),
            wk=_round_fp32r(W_qkv[:, C:2 * C][:, cols]),
            wv=_round_fp32r(W_qkv[:, 2 * C:][:, cols]),
            wo=_round_fp32r(W_o[cols, :]),
            bqs=np.ascontiguousarray(
                (b_qkv[:C][cols] * SCALE).reshape(HG, P).T, dtype=np.float32
            ),
            bks=np.ascontiguousarray(
                b_qkv[C:2 * C][cols].reshape(HG, P).T, dtype=np.float32
            ),
            bvt=np.ascontiguousarray(
                np.broadcast_to(b_qkv[2 * C:][cols], (P, GC)), dtype=np.float32
            ),
        )
    for core in range(NCORES):
        b, hg = core // 4, core % 4
        cols = slice(hg * GC, (hg + 1) * GC)
        m = dict(percore_hg[hg])
        m["xT"] = xTr[b]
        m["pkt"] = _round_fp32r(
            np.ascontiguousarray(past_k[b][:, cols].T).reshape(HG, P, PT)
        )
        m["pvn"] = _round_fp32r(past_v[b][:, cols])
        m["ones"] = ones
        in_maps.append(m)
    return in_maps


def _gather(results, x, past_k, past_v, b_o):
    y = np.empty((B, T, C), np.float32)
    k = np.empty((B, S, C), np.float32)
    v = np.empty((B, S, C), np.float32)
    for b in range(B):
        acc = results[b * 4]["yt"].copy()
        for hg in range(1, 4):
            acc += results[b * 4 + hg]["yt"]
        y[b] = acc.T + b_o[None, :]
        k[b, :PT] = past_k[b]
        v[b, :PT] = past_v[b]
        for hg in range(4):
            cols = slice(hg * GC, (hg + 1) * GC)
            ktn = results[b * 4 + hg]["ktn"]          # [HG, P, T]
            k[b, PT:, cols] = ktn.transpose(2, 0, 1).reshape(T, GC)
            v[b, PT:, cols] = results[b * 4 + hg]["vn"]
    return y, k, v


def run_device(in_maps, **kwargs):
    """Execute the SPMD program; returns BassKernelResults."""
    return run_bass_kernel_spmd(_get_nc(), in_maps, list(range(NCORES)), **kwargs)


def kernel(x, past_k, past_v, W_qkv, b_qkv, W_o, b_o):
    x = np.asarray(x, np.float32)
    past_k = np.asarray(past_k, np.float32)
    past_v = np.asarray(past_v, np.float32)
    W_qkv = np.asarray(W_qkv, np.float32)
    b_qkv = np.asarray(b_qkv, np.float32)
    W_o = np.asarray(W_o, np.float32)
    b_o = np.asarray(b_o, np.float32)

    in_maps = _prep_inputs(x, past_k, past_v, W_qkv, b_qkv, W_o, b_o)
    res = run_device(in_maps)
    return _gather(res.results, x, past_k, past_v, b_o)
